# revision 1
# baseline (speedup 1.0000x reference)
"""Expert-parallel batched FFN kernel for Trainium2 (8 NeuronCores).

Problem: y[e] = relu(x[e] @ fc1_w[e] + fc1_b[e]) @ fc2_w[e] + fc2_b[e]
with E=4, T=4096, D=2048, H=8192, fp32.

Sharding: expert-parallel × token-parallel. Core c handles expert e=c//2 and
token half h=c%2 (2048 tokens). Each core holds its expert's full weights, so
no collectives are needed.

Layout trick: both matmuls keep the contraction dim on SBUF partitions by
computing everything transposed:
    y1T[h,t] = W1[d,h].T-contract  (lhsT = W1 natural, rhs = xT)
    outT[d,t] = W2[h,d].T-contract (lhsT = W2 natural, rhs = y1T)
The host passes x pre-transposed (xT) and transposes the returned outT back,
so the device never transposes anything. Matmuls run as float32r (full PE
rate for moving dim >= 256). Layer-2 accumulation over all of H is done in
SBUF via DVE adds of per-h-chunk PSUM partials.
"""

import sys

for _p in ("/opt/trn_rl_repo", "/root/.axon_site/_ro/trn_rl_repo"):
    if _p not in sys.path:
        sys.path.append(_p)

import numpy as np

import concourse.bass as bass  # noqa: F401  (registers types)
import concourse.mybir as mybir
import concourse.tile as tile
from concourse import bacc
from concourse.bass_utils import run_bass_kernel_spmd

# Problem shapes (hardcoded per contract)
E, T, D, H = 4, 4096, 2048, 8192
NCORES = 8
TL = T * E // NCORES  # 2048 tokens per core
P = 128

# Tiling
TB = 512          # moving-dim (token) block per matmul
TPW = 1024        # tokens per weight pass (t-pair)
NTP = TL // TPW   # 2 weight passes
HC = 256          # h-chunk columns
NHC = H // HC     # 32 h-chunks
DK = D // P       # 16 k-subtiles for layer 1
HK = HC // P      # 2 k-subtiles per h-chunk for layer 2
DM = D // P       # 16 d-subtiles of the output

F32 = mybir.dt.float32
F32R = mybir.dt.float32r


def _build():
    nc = bacc.Bacc("TRN2", target_bir_lowering=False, debug=False,
                   num_devices=NCORES)

    xT = nc.dram_tensor("xT", [D, TL], F32, kind="ExternalInput")
    w1 = nc.dram_tensor("w1", [D, H], F32, kind="ExternalInput")
    b1 = nc.dram_tensor("b1", [H], F32, kind="ExternalInput")
    w2 = nc.dram_tensor("w2", [H, D], F32, kind="ExternalInput")
    b2 = nc.dram_tensor("b2", [D], F32, kind="ExternalInput")
    outT = nc.dram_tensor("outT", [D, TL], F32, kind="ExternalOutput")

    xT_r = xT[:].rearrange("(k p) t -> p k t", p=P).bitcast(F32R)    # [128,16,TL]
    w1_r = w1[:].rearrange("(k p) h -> p k h", p=P).bitcast(F32R)    # [128,16,H]
    w2_r = w2[:].rearrange("(k p) d -> p k d", p=P).bitcast(F32R)    # [128,64,D]
    outT_r = outT[:].rearrange("(s p) t -> p s t", p=P)              # [128,16,TL]
    b1_r = b1[:].rearrange("(s p) -> p s", p=P)                      # [128,64]
    b2_r = b2[:].rearrange("(s p) -> p s", p=P)                      # [128,16]

    with tile.TileContext(nc) as tc:
        with (
            tc.tile_pool(name="consts", bufs=1) as cpool,
            tc.tile_pool(name="xp", bufs=1) as xpool,
            tc.tile_pool(name="accp", bufs=1) as accpool,
            tc.tile_pool(name="w1p", bufs=2) as w1pool,
            tc.tile_pool(name="w2p", bufs=2) as w2pool,
            tc.tile_pool(name="y1p", bufs=2) as y1pool,
            tc.tile_pool(name="ps1", bufs=3, space="PSUM") as ps1pool,
            tc.tile_pool(name="ps2", bufs=4, space="PSUM") as ps2pool,
        ):
            b1_sb = cpool.tile([P, H // P], F32)
            b2_sb = cpool.tile([P, D // P], F32)
            nc.sync.dma_start(b1_sb[:], b1_r)
            nc.sync.dma_start(b2_sb[:], b2_r)

            for tp in range(NTP):
                t0 = tp * TPW
                x_sb = xpool.tile([P, DK, TPW], F32R)
                nc.sync.dma_start(x_sb[:], xT_r[:, :, t0:t0 + TPW])

                acc = accpool.tile([P, DM, TPW], F32)
                for s in range(DM):
                    nc.vector.tensor_copy(
                        acc[:, s, :],
                        b2_sb[:, s, None].to_broadcast((P, TPW)),
                    )

                for hc in range(NHC):
                    w1c = w1pool.tile([P, DK, HC], F32R)
                    nc.sync.dma_start(w1c[:], w1_r[:, :, hc * HC:(hc + 1) * HC])
                    w2c = w2pool.tile([P, HK, D], F32R)
                    nc.sync.dma_start(w2c[:], w2_r[:, hc * HK:(hc + 1) * HK, :])

                    for tb in range(TPW // TB):
                        tsl = slice(tb * TB, (tb + 1) * TB)
                        y1t = y1pool.tile([P, HK, TB], F32R)
                        # Layer 1: y1T chunk = relu(W1c.T @ xT + b1)
                        for m in range(HK):
                            ps = ps1pool.tile([P, TB], F32)
                            for k in range(DK):
                                nc.tensor.matmul(
                                    ps[:],
                                    w1c[:, k, m * P:(m + 1) * P],
                                    x_sb[:, k, tsl],
                                    start=(k == 0),
                                    stop=(k == DK - 1),
                                )
                            nc.scalar.activation(
                                y1t[:, m, :], ps[:],
                                mybir.ActivationFunctionType.Relu,
                                bias=b1_sb[:, hc * HK + m:hc * HK + m + 1],
                            )
                        # Layer 2: acc += W2c.T @ y1t
                        for n in range(DM):
                            ps = ps2pool.tile([P, TB], F32)
                            for k in range(HK):
                                nc.tensor.matmul(
                                    ps[:],
                                    w2c[:, k, n * P:(n + 1) * P],
                                    y1t[:, k, :],
                                    start=(k == 0),
                                    stop=(k == HK - 1),
                                )
                            nc.vector.tensor_add(
                                acc[:, n, tsl], acc[:, n, tsl], ps[:]
                            )

                nc.sync.dma_start(outT_r[:, :, t0:t0 + TPW], acc[:])

    nc.compile()
    return nc


_NC_CACHE = None


def _get_nc():
    global _NC_CACHE
    if _NC_CACHE is None:
        _NC_CACHE = _build()
    return _NC_CACHE


def _make_in_maps(x, fc1_w, fc1_b, fc2_w, fc2_b):
    in_maps = []
    for c in range(NCORES):
        e, th = divmod(c, NCORES // E)
        xs = x[e, th * TL:(th + 1) * TL, :]  # (TL, D)
        in_maps.append({
            "xT": np.ascontiguousarray(xs.T),
            "w1": np.ascontiguousarray(fc1_w[e]),
            "b1": np.ascontiguousarray(fc1_b[e, 0, :]),
            "w2": np.ascontiguousarray(fc2_w[e]),
            "b2": np.ascontiguousarray(fc2_b[e, 0, :]),
        })
    return in_maps


def run_spmd(in_maps, trace=False, **kwargs):
    """Compile (cached) and run the SPMD kernel; returns BassKernelResults."""
    nc = _get_nc()
    return run_bass_kernel_spmd(nc, in_maps, core_ids=list(range(NCORES)),
                                trace=trace, **kwargs)


def kernel(x, fc1_w, fc1_b, fc2_w, fc2_b):
    x = np.asarray(x, dtype=np.float32)
    fc1_w = np.asarray(fc1_w, dtype=np.float32)
    fc1_b = np.asarray(fc1_b, dtype=np.float32)
    fc2_w = np.asarray(fc2_w, dtype=np.float32)
    fc2_b = np.asarray(fc2_b, dtype=np.float32)

    in_maps = _make_in_maps(x, fc1_w, fc1_b, fc2_w, fc2_b)
    res = run_spmd(in_maps)

    out = np.empty((E, T, D), dtype=np.float32)
    for c in range(NCORES):
        e, th = divmod(c, NCORES // E)
        out[e, th * TL:(th + 1) * TL, :] = res.results[c]["outT"].T
    return out


# revision 2
# speedup vs baseline: 1.0095x; 1.0095x over previous
"""Expert-parallel batched FFN kernel for Trainium2 (8 NeuronCores).

Problem: y[e] = relu(x[e] @ fc1_w[e] + fc1_b[e]) @ fc2_w[e] + fc2_b[e]
with E=4, T=4096, D=2048, H=8192, fp32.

Sharding: expert-parallel × token-parallel. Core c handles expert e=c//2 and
token half h=c%2 (2048 tokens). Each core holds its expert's full weights, so
no collectives are needed.

Layout trick: both matmuls keep the contraction dim on SBUF partitions by
computing everything transposed:
    y1T[h,t] = W1[d,h].T-contract  (lhsT = W1 natural, rhs = xT)
    outT[d,t] = W2[h,d].T-contract (lhsT = W2 natural, rhs = y1T)
The host passes x pre-transposed (xT) and transposes the returned outT back,
so the device never transposes anything. Matmuls run as float32r (full PE
rate for moving dim >= 256). Layer-2 accumulation over all of H is done in
SBUF via DVE adds of per-h-chunk PSUM partials.
"""

import sys

for _p in ("/opt/trn_rl_repo", "/root/.axon_site/_ro/trn_rl_repo"):
    if _p not in sys.path:
        sys.path.append(_p)

import numpy as np

import concourse.bass as bass  # noqa: F401  (registers types)
import concourse.mybir as mybir
import concourse.tile as tile
from concourse import bacc
from concourse.bass_utils import run_bass_kernel_spmd

# Problem shapes (hardcoded per contract)
E, T, D, H = 4, 4096, 2048, 8192
NCORES = 8
TL = T * E // NCORES  # 2048 tokens per core
P = 128

# Tiling
TB = 512          # moving-dim (token) block per matmul
TPW = 1024        # tokens per weight pass (t-pair)
NTP = TL // TPW   # 2 weight passes
HC = 256          # h-chunk columns
NHC = H // HC     # 32 h-chunks
DK = D // P       # 16 k-subtiles for layer 1
HK = HC // P      # 2 k-subtiles per h-chunk for layer 2
DM = D // P       # 16 d-subtiles of the output

F32 = mybir.dt.float32
F32R = mybir.dt.float32r


def _build():
    nc = bacc.Bacc("TRN2", target_bir_lowering=False, debug=False,
                   num_devices=NCORES)

    xT = nc.dram_tensor("xT", [D, TL], F32, kind="ExternalInput")
    w1 = nc.dram_tensor("w1", [D, H], F32, kind="ExternalInput")
    b1 = nc.dram_tensor("b1", [H], F32, kind="ExternalInput")
    w2 = nc.dram_tensor("w2", [H, D], F32, kind="ExternalInput")
    b2 = nc.dram_tensor("b2", [D], F32, kind="ExternalInput")
    outT = nc.dram_tensor("outT", [D, TL], F32, kind="ExternalOutput")

    xT_r = xT[:].rearrange("(k p) t -> p k t", p=P).bitcast(F32R)    # [128,16,TL]
    w1_r = w1[:].rearrange("(k p) h -> p k h", p=P).bitcast(F32R)    # [128,16,H]
    w2_r = w2[:].rearrange("(k p) d -> p k d", p=P).bitcast(F32R)    # [128,64,D]
    outT_r = outT[:].rearrange("(s p) t -> p s t", p=P)              # [128,16,TL]
    b1_r = b1[:].rearrange("(s p) -> p s", p=P)                      # [128,64]
    b2_r = b2[:].rearrange("(s p) -> p s", p=P)                      # [128,16]

    with tile.TileContext(nc) as tc:
        with (
            tc.tile_pool(name="consts", bufs=1) as cpool,
            tc.tile_pool(name="xp", bufs=DK) as xpool,
            tc.tile_pool(name="accp", bufs=DM) as accpool,
            tc.tile_pool(name="w1p", bufs=2) as w1pool,
            tc.tile_pool(name="w2p", bufs=2) as w2pool,
            tc.tile_pool(name="y1p", bufs=2) as y1pool,
            tc.tile_pool(name="ps1", bufs=3, space="PSUM") as ps1pool,
            tc.tile_pool(name="ps2", bufs=4, space="PSUM") as ps2pool,
        ):
            b1_sb = cpool.tile([P, H // P], F32)
            b2_sb = cpool.tile([P, D // P], F32)
            nc.sync.dma_start(b1_sb[:], b1_r)
            nc.sync.dma_start(b2_sb[:], b2_r)

            for tp in range(NTP):
                t0 = tp * TPW
                # Per-k xT tiles: fine-grained loads so the first matmul
                # chain starts as soon as slice k=0 lands, and the next
                # t-pair's loads recycle slots per-k (prefetch across the
                # pair boundary).
                xs = []
                for k in range(DK):
                    x_k = xpool.tile([P, TPW], F32R, tag="xk")
                    nc.sync.dma_start(x_k[:], xT_r[:, k, t0:t0 + TPW])
                    xs.append(x_k)

                # Per-s output accumulators (bias-initialised); per-s slot
                # recycling lets drains/inits pipeline across pairs.
                accs = []
                for s in range(DM):
                    a_s = accpool.tile([P, TPW], F32, tag="accs")
                    nc.vector.tensor_copy(
                        a_s[:], b2_sb[:, s, None].to_broadcast((P, TPW))
                    )
                    accs.append(a_s)

                for hc in range(NHC):
                    w1c = w1pool.tile([P, DK, HC], F32R)
                    for k in range(DK):
                        nc.sync.dma_start(
                            w1c[:, k, :], w1_r[:, k, hc * HC:(hc + 1) * HC]
                        )
                    w2c = w2pool.tile([P, HK, D], F32R)
                    for k in range(HK):
                        nc.sync.dma_start(
                            w2c[:, k, :], w2_r[:, hc * HK + k, :]
                        )

                    for tb in range(TPW // TB):
                        tsl = slice(tb * TB, (tb + 1) * TB)
                        y1t = y1pool.tile([P, HK, TB], F32R)
                        # Layer 1: y1T chunk = relu(W1c.T @ xT + b1)
                        for m in range(HK):
                            ps = ps1pool.tile([P, TB], F32)
                            for k in range(DK):
                                nc.tensor.matmul(
                                    ps[:],
                                    w1c[:, k, m * P:(m + 1) * P],
                                    xs[k][:, tsl],
                                    start=(k == 0),
                                    stop=(k == DK - 1),
                                )
                            nc.scalar.activation(
                                y1t[:, m, :], ps[:],
                                mybir.ActivationFunctionType.Relu,
                                bias=b1_sb[:, hc * HK + m:hc * HK + m + 1],
                            )
                        # Layer 2: acc += W2c.T @ y1t
                        for n in range(DM):
                            ps = ps2pool.tile([P, TB], F32)
                            for k in range(HK):
                                nc.tensor.matmul(
                                    ps[:],
                                    w2c[:, k, n * P:(n + 1) * P],
                                    y1t[:, k, :],
                                    start=(k == 0),
                                    stop=(k == HK - 1),
                                )
                            nc.vector.tensor_add(
                                accs[n][:, tsl], accs[n][:, tsl], ps[:]
                            )

                for s in range(DM):
                    nc.sync.dma_start(outT_r[:, s, t0:t0 + TPW], accs[s][:])

    nc.compile()
    return nc


_NC_CACHE = None


def _get_nc():
    global _NC_CACHE
    if _NC_CACHE is None:
        _NC_CACHE = _build()
    return _NC_CACHE


def _make_in_maps(x, fc1_w, fc1_b, fc2_w, fc2_b):
    in_maps = []
    for c in range(NCORES):
        e, th = divmod(c, NCORES // E)
        xs = x[e, th * TL:(th + 1) * TL, :]  # (TL, D)
        in_maps.append({
            "xT": np.ascontiguousarray(xs.T),
            "w1": np.ascontiguousarray(fc1_w[e]),
            "b1": np.ascontiguousarray(fc1_b[e, 0, :]),
            "w2": np.ascontiguousarray(fc2_w[e]),
            "b2": np.ascontiguousarray(fc2_b[e, 0, :]),
        })
    return in_maps


def run_spmd(in_maps, trace=False, **kwargs):
    """Compile (cached) and run the SPMD kernel; returns BassKernelResults."""
    nc = _get_nc()
    return run_bass_kernel_spmd(nc, in_maps, core_ids=list(range(NCORES)),
                                trace=trace, **kwargs)


def kernel(x, fc1_w, fc1_b, fc2_w, fc2_b):
    x = np.asarray(x, dtype=np.float32)
    fc1_w = np.asarray(fc1_w, dtype=np.float32)
    fc1_b = np.asarray(fc1_b, dtype=np.float32)
    fc2_w = np.asarray(fc2_w, dtype=np.float32)
    fc2_b = np.asarray(fc2_b, dtype=np.float32)

    in_maps = _make_in_maps(x, fc1_w, fc1_b, fc2_w, fc2_b)
    res = run_spmd(in_maps)

    out = np.empty((E, T, D), dtype=np.float32)
    for c in range(NCORES):
        e, th = divmod(c, NCORES // E)
        out[e, th * TL:(th + 1) * TL, :] = res.results[c]["outT"].T
    return out
